# revision 1
# baseline (speedup 1.0000x reference)
"""Trainium2 Bass kernel v2 for nn_Attention_48610439856262.

Gated attention block:
    qkv = x @ W_qkv ; gate = x @ W_gate ; s = e @ W_s (added to k)
    attn = softmax(q @ (k+s).T * D**-0.5) ; out = (attn @ v) * gate
    y = out @ W_proj + b_proj

Sharding (8 cores, tensor-parallel over heads): core c owns heads
{2c, 2c+1}.  Each core computes its 128 feature columns for q/k+s/v/gate,
runs attention for 2 heads, and writes the partial projection
y_c = gated_c @ W_proj[128c:128c+128, :]; the host sums partials + b_proj.

v2 changes vs v1 (226.8us baseline):
  * bf16 activations/weights everywhere (same PE rate as f32r, half the
    DMA + SBUF traffic; metric 2.7e-3 vs 2e-2 budget).
  * k+s fused into ONE PSUM accumulation (16 matmuls) - no ktmp/add.
  * attn@v in fp8e4 + DoubleRow (contraction 256/instr): pt (exp output)
    and v quantized to e4m3; numpy-sim metric 1.45e-2 < 2e-2.
  * v_aug layout [v(64) | ones] per head -> denominator lands on PSUM
    partition 64; head0 needs no partition-shift DMA (h1 still does).
  * batch-pipelined: A(b0) | B(b0)+A(b1) | B(b1)+proj(b0) | proj(b1),
    with input DMA prefetch ahead of PE.
  * copies distributed DVE/gpsimd so ACT does exp only (ACT is the
    phase-B bottleneck at ~1.04us per [128,1024] exp tile).
"""

import numpy as np

B, N, C, H, D = 2, 2048, 1024, 16, 64
T = B * N              # 4096 tokens
NCORES = 8
F = 128                # feature columns per core (2 heads x 64)
SCALE = D ** -0.5
KC = C // 128          # 8 contraction chunks
TB = T // 128          # 32 token blocks
NB = N // 128          # 16 key blocks per sequence

FP8_ATTNV = True       # fp8e4+DoubleRow attn@v (else bf16)

_cache: dict = {}


def _build_program(reps=1):
    import concourse.bacc as bacc
    import concourse.tile as tile
    from concourse import mybir
    from concourse.masks import make_identity

    f32 = mybir.dt.float32
    bf16 = mybir.dt.bfloat16
    fp8 = mybir.dt.float8e4
    vdt = fp8 if FP8_ATTNV else bf16
    DR = mybir.MatmulPerfMode.DoubleRow
    Exp = mybir.ActivationFunctionType.Exp

    nc = bacc.Bacc("TRN2", target_bir_lowering=False, debug=False,
                   num_devices=NCORES)

    xT = nc.dram_tensor("xT", [C, T], bf16, kind="ExternalInput").ap()
    eT = nc.dram_tensor("eT", [C, T], bf16, kind="ExternalInput").ap()
    wq = nc.dram_tensor("wq", [C, F], bf16, kind="ExternalInput").ap()
    wk = nc.dram_tensor("wk", [C, F], bf16, kind="ExternalInput").ap()
    wv = nc.dram_tensor("wv", [C, F], bf16, kind="ExternalInput").ap()
    ws = nc.dram_tensor("ws", [C, F], bf16, kind="ExternalInput").ap()
    wg = nc.dram_tensor("wg", [C, F], bf16, kind="ExternalInput").ap()
    wp = nc.dram_tensor("wp", [F, C], bf16, kind="ExternalInput").ap()
    y = nc.dram_tensor("y", [T, C], bf16, kind="ExternalOutput").ap()

    with tile.TileContext(nc) as tc:
        with tc.tile_pool(name="persist", bufs=1) as persist, \
             tc.tile_pool(name="psum", bufs=1, space="PSUM") as psum, \
             tc.tile_pool(name="xa", bufs=4) as xa_pool, \
             tc.tile_pool(name="ea", bufs=4) as ea_pool, \
             tc.tile_pool(name="vt", bufs=3) as vt_pool, \
             tc.tile_pool(name="pt", bufs=3) as pt_pool, \
             tc.tile_pool(name="small", bufs=4) as small, \
             tc.tile_pool(name="yout", bufs=4) as y_pool:
            # Weights, contraction-chunked: [128 k-part, KC, 128 cols]
            w_sb = {}
            for name, src in (("wq", wq), ("wk", wk), ("wv", wv),
                              ("ws", ws), ("wg", wg)):
                t_ = persist.tile([128, KC, F], bf16, tag=name, name=f"w_{name}")
                nc.sync.dma_start(out=t_,
                                  in_=src.rearrange("(k p) f -> p k f", p=128))
                w_sb[name] = t_
            wp_sb = persist.tile([F, C], bf16, tag="wp")
            nc.sync.dma_start(out=wp_sb, in_=wp)
            ident = persist.tile([128, 128], bf16, tag="ident")
            make_identity(nc, ident)

            qT_s = persist.tile([128, T], bf16, tag="qT")
            kpsT_s = persist.tile([128, T], bf16, tag="kpsT")
            gT_s = persist.tile([128, T], bf16, tag="gT")
            gatedT_s = persist.tile([128, T], bf16, tag="gatedT")
            # v_aug per head: [v(64) | ones | pad(15)] x 2 heads -> 160/blk.
            # attn@v out partitions 0-63 = head dims, partition 64 = softmax
            # denominator.  160B block stride satisfies DoubleRow step%16.
            v_s = persist.tile([128, TB, 160], vdt, tag="v")
            nc.vector.memset(v_s[:, :, 64], 1.0)
            nc.vector.memset(v_s[:, :, 144], 1.0)

            # PSUM (8 banks): scores [128,1024]x2bufs = 4 | psv0,psv1 = 2 |
            # pework [128,512]x2bufs = 2 (phase-A acc + transposes + proj)

            chunk_state = {}

            def phase_a_part(t, part):
                """One quarter of chunk t's projections (fine-grained so PE
                can interleave it into ACT-bound attention slack).
                part 0: DMAs + q | 1: k+s | 2: gate | 3: v + transposes."""
                sl = slice(t * 512, (t + 1) * 512)
                if part == 0:
                    xt = xa_pool.tile([128, KC, 512], bf16, tag="xt",
                                      name=f"xt{t}")
                    nc.sync.dma_start(
                        out=xt,
                        in_=xT[:, sl].rearrange("(k p) t -> p k t", p=128))
                    et = ea_pool.tile([128, KC, 512], bf16, tag="et",
                                      name=f"et{t}")
                    nc.sync.dma_start(
                        out=et,
                        in_=eT[:, sl].rearrange("(k p) t -> p k t", p=128))
                    chunk_state[t] = (xt, et)
                    acc = psum.tile([128, 512], f32, tag="pework", bufs=2,
                                    name="acc_q")
                    for k in range(KC):
                        nc.tensor.matmul(acc, w_sb["wq"][:, k, :],
                                         xt[:, k, :],
                                         start=(k == 0), stop=(k == KC - 1))
                    nc.vector.tensor_copy(qT_s[:, sl], acc)
                elif part == 1:
                    xt, et = chunk_state[t]
                    acc = psum.tile([128, 512], f32, tag="pework", bufs=2,
                                    name="acc_ks")
                    for k in range(KC):
                        nc.tensor.matmul(acc, w_sb["wk"][:, k, :],
                                         xt[:, k, :],
                                         start=(k == 0), stop=False)
                    for k in range(KC):
                        nc.tensor.matmul(acc, w_sb["ws"][:, k, :],
                                         et[:, k, :],
                                         start=False, stop=(k == KC - 1))
                    nc.vector.tensor_copy(kpsT_s[:, sl], acc)
                elif part == 2:
                    xt, _ = chunk_state[t]
                    acc = psum.tile([128, 512], f32, tag="pework", bufs=2,
                                    name="acc_g")
                    for k in range(KC):
                        nc.tensor.matmul(acc, w_sb["wg"][:, k, :],
                                         xt[:, k, :],
                                         start=(k == 0), stop=(k == KC - 1))
                    nc.vector.tensor_copy(gT_s[:, sl], acc)
                else:
                    xt, _ = chunk_state.pop(t)
                    acc = psum.tile([128, 512], f32, tag="pework", bufs=2,
                                    name="acc_v")
                    for k in range(KC):
                        nc.tensor.matmul(acc, w_sb["wv"][:, k, :],
                                         xt[:, k, :],
                                         start=(k == 0), stop=(k == KC - 1))
                    vt_tmp = vt_pool.tile([128, 512], bf16, tag="vt")
                    nc.vector.tensor_copy(vt_tmp, acc)
                    for j in range(4):
                        tb = t * 4 + j
                        vT = vt_pool.tile([128, 2, 64], bf16, tag="vtT",
                                          name="vT")
                        nc.sync.dma_start_transpose(
                            vT[:, 0, :], vt_tmp[0:64, j * 128:(j + 1) * 128])
                        nc.sync.dma_start_transpose(
                            vT[:, 1, :],
                            vt_tmp[64:128, j * 128:(j + 1) * 128])
                        nc.vector.tensor_copy(
                            v_s[:, tb, :].rearrange("p (h c) -> p h c",
                                                    h=2)[:, :, 0:64], vT)

            def phase_a_chunk(t):
                for part in range(4):
                    phase_a_part(t, part)

            spills = {}

            def attn_half(b, nh, h, part):
                """Half-section (8 key blocks) of attention for (batch b,
                query half nh, head h).  part 0 accumulates attn@v for keys
                0-1023 and spills the PSUM accumulator to SBUF (frees the
                psv banks so other sections' exps can start early); part 1
                does keys 1024-2047, merges the spill, and normalizes."""
                hsl = slice(h * 64, h * 64 + 64)
                voff = h * 80
                qbase = b * N + nh * 1024
                psv = [psum.tile([65, 512], f32, tag=f"psv{jj}",
                                 name=f"psv{jj}") for jj in range(2)]
                mb0 = part * 8
                for mb in range(mb0, mb0 + 8):
                    msl = slice(b * N + mb * 128, b * N + mb * 128 + 128)
                    ps = psum.tile([128, 1024], f32, tag="scores", bufs=2,
                                   name="scores")
                    if mb % 2 == 0:
                        ptt = pt_pool.tile([128, 2, 1024], vdt, tag="pt",
                                           name="ptt")
                    for jj in range(2):
                        nsl = slice(qbase + jj * 512, qbase + (jj + 1) * 512)
                        nc.tensor.matmul(ps[:, jj * 512:(jj + 1) * 512],
                                         kpsT_s[hsl, msl], qT_s[hsl, nsl],
                                         start=True, stop=True)
                    nc.scalar.activation(ptt[:, mb % 2, :], ps, Exp,
                                         scale=SCALE)
                    if FP8_ATTNV:
                        if mb % 2 == 1:
                            pr = b * NB + mb - 1
                            for jj in range(2):
                                nc.tensor.matmul(
                                    psv[jj],
                                    v_s[:, pr:pr + 2, voff:voff + 65],
                                    ptt[:, :, jj * 512:(jj + 1) * 512],
                                    start=(mb == mb0 + 1),
                                    stop=(mb == mb0 + 7),
                                    perf_mode=DR)
                    else:
                        for jj in range(2):
                            nc.tensor.matmul(
                                psv[jj],
                                v_s[:, b * NB + mb, voff:voff + 65],
                                ptt[:, mb % 2, jj * 512:(jj + 1) * 512],
                                start=(mb == mb0), stop=(mb == mb0 + 7))
                if part == 0:
                    for jj in range(2):
                        sp = small.tile([65, 512], f32, tag="spill", bufs=8,
                                        name="spill")
                        nc.vector.tensor_copy(sp, psv[jj])
                        spills[(b, nh, h, jj)] = sp
                    return
                for jj in range(2):
                    nsl = slice(qbase + jj * 512, qbase + (jj + 1) * 512)
                    sacc = small.tile([65, 512], f32, tag="sacc")
                    nc.vector.tensor_add(sacc, spills.pop((b, nh, h, jj)),
                                         psv[jj])
                    # partition_broadcast only works from partition 0 on HW
                    # (the DSP routine ignores the AP's partition offset), so
                    # DMA the denominator row down to partition 0 first.
                    d0 = small.tile([1, 512], f32, tag="d0")
                    nc.sync.dma_start(out=d0, in_=sacc[64:65, :])
                    rs = small.tile([1, 512], f32, tag="rs")
                    nc.vector.reciprocal(rs, d0)
                    rb = small.tile([64, 512], f32, tag="rb")
                    nc.gpsimd.partition_broadcast(rb, rs)
                    tmp = small.tile([128, 512], bf16, tag="tmp")
                    nc.gpsimd.tensor_mul(tmp[0:64, :], sacc[0:64, :], rb)
                    if h == 0:
                        nc.gpsimd.tensor_mul(gatedT_s[hsl, nsl], tmp[0:64, :],
                                             gT_s[hsl, nsl])
                    else:
                        tmp2 = small.tile([128, 512], bf16, tag="tmp2")
                        nc.sync.dma_start(out=tmp2[64:128, :],
                                          in_=tmp[0:64, :])
                        nc.gpsimd.tensor_mul(gatedT_s[hsl, nsl],
                                             tmp2[64:128, :], gT_s[hsl, nsl])

            def proj_blocks(tbs):
                """Partial projection y[tb*128:...] for token blocks tbs.
                One merged bf16 y DMA per block (PSUM drained by DVE; only
                DVE/ACT can read PSUM and ACT is saturated by exp)."""
                for tb in tbs:
                    yt = y_pool.tile([128, 1024], bf16, tag="yt")
                    for j in range(2):
                        py = psum.tile([128, 512], f32, tag="pework", bufs=2,
                                       name="proj")
                        nc.tensor.matmul(py,
                                         gatedT_s[:, tb * 128:(tb + 1) * 128],
                                         wp_sb[:, j * 512:(j + 1) * 512],
                                         start=True, stop=True)
                        nc.vector.tensor_copy(yt[:, j * 512:(j + 1) * 512],
                                              py)
                    nc.sync.dma_start(out=y[tb * 128:(tb + 1) * 128, :],
                                      in_=yt)

            # Software-pipelined across reps: exp (ACT) is the phase-B
            # bottleneck, so issue order keeps ACT fed continuously while PE
            # fills ACT-bound slack with fine-grained phase-A parts (2 per
            # attention half, covering b1's chunks then the NEXT rep's b0
            # chunks) plus projections (2 blocks per half; b1 blocks of rep
            # r run early in rep r+1).
            halves = [(0, 0, 0, 0), (0, 0, 1, 0), (0, 1, 0, 0), (0, 1, 1, 0),
                      (0, 0, 0, 1), (0, 0, 1, 1), (0, 1, 0, 1), (0, 1, 1, 1),
                      (1, 0, 0, 0), (1, 0, 1, 0), (1, 1, 0, 0), (1, 1, 1, 0),
                      (1, 0, 0, 1), (1, 0, 1, 1), (1, 1, 0, 1), (1, 1, 1, 1)]
            phase_a_chunk(0)
            phase_a_chunk(1)
            phase_a_chunk(2)
            phase_a_chunk(3)
            for _rep in range(reps):
                last = _rep == reps - 1
                a_parts = [(c, p) for c in (4, 5, 6, 7) for p in range(4)]
                if not last:
                    a_parts += [(c, p) for c in (0, 1, 2, 3) for p in range(4)]
                else:
                    a_parts += [None] * 16
                # proj bunches: prev rep's b1 blocks early, b0 blocks once
                # their gatedT halves land; b1 blocks deferred to next rep
                projs = {8: range(0, 8), 10: range(8, 16)}
                if _rep > 0:
                    projs[3] = range(16, 24)
                    projs[5] = range(24, 32)
                for i, (b, nh, h, part) in enumerate(halves):
                    attn_half(b, nh, h, part)
                    for cp in a_parts[2 * i:2 * i + 2]:
                        if cp is not None:
                            phase_a_part(*cp)
                    proj_blocks(projs.get(i, ()))
            proj_blocks(range(16, 32))

    nc.compile()
    return nc


def _get_nc():
    if "nc" not in _cache:
        _cache["nc"] = _build_program()
    return _cache["nc"]


def _get_exec():
    """Compile once; cache a persistent sharded executable."""
    if "exec" in _cache:
        return _cache["exec"]
    import jax
    from jax.experimental.shard_map import shard_map
    from jax.sharding import Mesh, PartitionSpec
    from concourse import mybir
    from concourse.bass2jax import (_bass_exec_p, install_neuronx_cc_hook,
                                    partition_id_tensor)

    nc = _get_nc()
    install_neuronx_cc_hook()
    partition_name = (nc.partition_id_tensor.name
                      if nc.partition_id_tensor else None)
    in_names, out_names, out_avals = [], [], []
    for alloc in nc.m.functions[0].allocations:
        if not isinstance(alloc, mybir.MemoryLocationSet):
            continue
        name = alloc.memorylocations[0].name
        if alloc.kind == "ExternalInput":
            if name != partition_name:
                in_names.append(name)
        elif alloc.kind == "ExternalOutput":
            out_names.append(name)
            out_avals.append(jax.core.ShapedArray(
                tuple(alloc.tensor_shape), mybir.dt.np(alloc.dtype)))
    n_params, n_outs = len(in_names), len(out_names)
    bind_in_names = tuple(in_names + out_names +
                          ([partition_name] if partition_name else []))

    def _body(*args):
        operands = list(args)
        if partition_name is not None:
            operands.append(partition_id_tensor())
        outs = _bass_exec_p.bind(
            *operands,
            out_avals=tuple(out_avals),
            in_names=bind_in_names,
            out_names=tuple(out_names),
            lowering_input_output_aliases=(),
            sim_require_finite=True,
            sim_require_nnan=True,
            nc=nc,
        )
        return tuple(outs)

    devices = jax.devices()[:NCORES]
    mesh = Mesh(np.asarray(devices), ("core",))
    in_specs = (PartitionSpec("core"),) * (n_params + n_outs)
    out_specs = (PartitionSpec("core"),) * n_outs
    sharded = jax.jit(shard_map(_body, mesh=mesh, in_specs=in_specs,
                                out_specs=out_specs, check_rep=False),
                      keep_unused=True)
    zeros_dev = [
        jax.device_put(
            np.zeros((NCORES * a.shape[0], *a.shape[1:]), a.dtype),
            jax.sharding.NamedSharding(mesh, PartitionSpec("core")))
        for a in out_avals]
    reduce_fn = jax.jit(
        lambda a: a.reshape(NCORES, T, C).astype(jax.numpy.float32).sum(axis=0))
    ex = {"fn": sharded, "in_names": in_names, "out_names": out_names,
          "out_avals": out_avals, "mesh": mesh, "zeros_dev": zeros_dev,
          "spec": PartitionSpec("core"), "reduce": reduce_fn}
    _cache["exec"] = ex
    return ex


def _make_in_maps(x, e, W_qkv, W_s, W_gate, W_proj):
    import ml_dtypes
    bf = ml_dtypes.bfloat16
    xT = np.ascontiguousarray(
        np.asarray(x, np.float32).reshape(T, C).T).astype(bf)
    eT = np.ascontiguousarray(
        np.asarray(e, np.float32).reshape(T, C).T).astype(bf)
    in_maps = []
    for c in range(NCORES):
        fs = slice(F * c, F * (c + 1))
        in_maps.append({
            "xT": xT,
            "eT": eT,
            "wq": np.ascontiguousarray(W_qkv[:, fs]).astype(bf),
            "wk": np.ascontiguousarray(W_qkv[:, C:][:, fs]).astype(bf),
            "wv": np.ascontiguousarray(W_qkv[:, 2 * C:][:, fs]).astype(bf),
            "ws": np.ascontiguousarray(W_s[:, fs]).astype(bf),
            "wg": np.ascontiguousarray(W_gate[:, fs]).astype(bf),
            "wp": np.ascontiguousarray(W_proj[fs, :]).astype(bf),
        })
    return in_maps


def kernel(x, e, W_qkv, W_s, W_gate, W_proj, b_proj):
    ex = _get_exec()
    in_maps = _make_in_maps(np.asarray(x), np.asarray(e), np.asarray(W_qkv),
                            np.asarray(W_s), np.asarray(W_gate),
                            np.asarray(W_proj))
    concat_in = [
        np.concatenate([np.asarray(in_maps[c][name])
                        for c in range(NCORES)], axis=0)
        for name in ex["in_names"]]
    out = ex["fn"](*concat_in, *ex["zeros_dev"])
    iy = ex["out_names"].index("y")
    y_sum = np.asarray(ex["reduce"](out[iy]))   # cross-core partial sum
    y_sum = y_sum + np.asarray(b_proj, dtype=np.float32)
    return y_sum.reshape(B, N, C).astype(np.float32)



# revision 2
# speedup vs baseline: 1.1841x; 1.1841x over previous
"""Trainium2 Bass kernel v3 for nn_Attention_48610439856262.

Gated attention block:
    qkv = x @ W_qkv ; gate = x @ W_gate ; s = e @ W_s (added to k)
    attn = softmax(q @ (k+s).T * D**-0.5) ; out = (attn @ v) * gate
    y = out @ W_proj + b_proj

Sharding (8 cores, tensor-parallel over heads): core c owns heads
{2c, 2c+1}.  Each core computes its 128 feature columns for q/k+s/v/gate,
runs attention for 2 heads, and writes the partial projection
y_c = gated_c @ W_proj[128c:128c+128, :]; the host sums partials + b_proj.

v3 changes vs v2 (427us HW steady-state):
  * scores via 2x PE row tiling: h0 (SBUF partitions 0-63) and h1 (64-127)
    issue back-to-back as tile_position (0,0)/(64,0) matmuls into separate
    PSUM banks of one [128, 2, 512] tile -> concurrent on the array.
  * one exp covers both heads' scores tile ([128, 1024] ACT instr).
  * attnv accumulates the full 2048-key range into psv (per section
    (b, nh, jj)) -> no spill/merge round-trip.
  * phase A runs chunk PAIRS sharing each stationary weight load (halves
    LDWEIGHTS traffic, which HW pays serially).
  * proj transposed: stationary = W_proj column chunk (loaded once per 4-8
    moving token groups), output yT [C, T]; host transposes.
"""

import numpy as np

B, N, C, H, D = 2, 2048, 1024, 16, 64
T = B * N              # 4096 tokens
NCORES = 8
F = 128                # feature columns per core (2 heads x 64)
KC = C // 128          # 8 contraction chunks
TB = T // 128          # 32 token blocks
NB = N // 128          # 16 key blocks per sequence
SCALE = D ** -0.5

_cache: dict = {}


def _build_program(reps=1, stage=4, proj_t=False, norm="dve", act2=False):
    import concourse.bacc as bacc
    import concourse.tile as tile
    from concourse import mybir

    f32 = mybir.dt.float32
    bf16 = mybir.dt.bfloat16
    fp8 = mybir.dt.float8e4
    DR = mybir.MatmulPerfMode.DoubleRow
    Exp = mybir.ActivationFunctionType.Exp

    nc = bacc.Bacc("TRN2", target_bir_lowering=False, debug=False,
                   num_devices=NCORES)

    xT = nc.dram_tensor("xT", [C, T], bf16, kind="ExternalInput").ap()
    eT = nc.dram_tensor("eT", [C, T], bf16, kind="ExternalInput").ap()
    wq = nc.dram_tensor("wq", [C, F], bf16, kind="ExternalInput").ap()
    wk = nc.dram_tensor("wk", [C, F], bf16, kind="ExternalInput").ap()
    wv = nc.dram_tensor("wv", [C, F], bf16, kind="ExternalInput").ap()
    ws = nc.dram_tensor("ws", [C, F], bf16, kind="ExternalInput").ap()
    wg = nc.dram_tensor("wg", [C, F], bf16, kind="ExternalInput").ap()
    wp = nc.dram_tensor("wp", [F, C], bf16, kind="ExternalInput").ap()
    y = nc.dram_tensor("y", [C, T] if proj_t else [T, C], bf16,
                       kind="ExternalOutput").ap()

    with tile.TileContext(nc) as tc:
        with tc.tile_pool(name="persist", bufs=1) as persist, \
             tc.tile_pool(name="psum", bufs=1, space="PSUM") as psum, \
             tc.tile_pool(name="xa", bufs=4) as xa_pool, \
             tc.tile_pool(name="ea", bufs=4) as ea_pool, \
             tc.tile_pool(name="vt", bufs=3) as vt_pool, \
             tc.tile_pool(name="pt", bufs=3) as pt_pool, \
             tc.tile_pool(name="small", bufs=4) as small, \
             tc.tile_pool(name="yout", bufs=4) as y_pool:
            # Weights, contraction-chunked: [128 k-part, KC, 128 cols]
            w_sb = {}
            for name, src in (("wq", wq), ("wk", wk), ("wv", wv),
                              ("ws", ws), ("wg", wg)):
                t_ = persist.tile([128, KC, F], bf16, tag=name, name=f"w_{name}")
                nc.sync.dma_start(out=t_,
                                  in_=src.rearrange("(k p) f -> p k f", p=128))
                w_sb[name] = t_
            wp_sb = persist.tile([F, C], bf16, tag="wp")
            nc.sync.dma_start(out=wp_sb, in_=wp)

            qT_s = persist.tile([128, T], bf16, tag="qT")
            kpsT_s = persist.tile([128, T], bf16, tag="kpsT")
            gT_s = persist.tile([128, T], bf16, tag="gT")
            gatedT_s = persist.tile([128, T], bf16, tag="gatedT")
            # v_aug per head: [v(64) | ones | pad(15)] x 2 heads -> 160/blk.
            # attn@v out partitions 0-63 = head dims, partition 64 = softmax
            # denominator.  160B block stride satisfies DoubleRow step%16.
            v_s = persist.tile([128, TB, 160], fp8, tag="v")
            nc.vector.memset(v_s[:, :, 64], 1.0)
            nc.vector.memset(v_s[:, :, 144], 1.0)

            # PSUM (8 banks): scores [128,2,512] x 2 bufs = 4 | psv0,psv1 = 2
            # | pework [128,512] x 2 bufs = 2 (phase-A acc + proj)

            chunk_state = {}

            def phase_a_dma(t):
                """Prefetch chunk t's x/e slabs (issued ahead of the PE
                chains so the first matmul never waits on HBM)."""
                sl = slice(t * 512, (t + 1) * 512)
                xt = xa_pool.tile([128, KC, 512], bf16, tag="xt",
                                  name=f"xt{t}")
                nc.sync.dma_start(
                    out=xt,
                    in_=xT[:, sl].rearrange("(k p) t -> p k t", p=128))
                et = ea_pool.tile([128, KC, 512], bf16, tag="et",
                                  name=f"et{t}")
                nc.sync.dma_start(
                    out=et,
                    in_=eT[:, sl].rearrange("(k p) t -> p k t", p=128))
                chunk_state[t] = (xt, et)

            def phase_a_part(t, part):
                """One quarter of chunk t's projections.  Chains stay
                bank-contiguous (HW pays heavily for interleaved PSUM
                accumulation groups).  part 0: q | 1: k+s | 2: gate
                | 3: v + transposes."""
                sl = slice(t * 512, (t + 1) * 512)
                wname, dst = (("wq", qT_s), ("wk", kpsT_s), ("wg", gT_s),
                              ("wv", None))[part]
                acc = psum.tile([128, 512], f32, tag="pework", bufs=2,
                                name=f"acc_p{part}")
                xt, et = chunk_state[t]
                for k in range(KC):
                    nc.tensor.matmul(acc, w_sb[wname][:, k, :], xt[:, k, :],
                                     start=(k == 0),
                                     stop=(part != 1 and k == KC - 1))
                if part == 1:
                    for k in range(KC):
                        nc.tensor.matmul(acc, w_sb["ws"][:, k, :],
                                         et[:, k, :],
                                         start=False, stop=(k == KC - 1))
                if dst is not None:
                    nc.vector.tensor_copy(dst[:, sl], acc)
                else:
                    vt_tmp = vt_pool.tile([128, 512], bf16, tag="vt")
                    nc.vector.tensor_copy(vt_tmp, acc)
                    for j in range(4):
                        tb = t * 4 + j
                        vT = vt_pool.tile([128, 2, 64], bf16, tag="vtT",
                                          name="vT")
                        nc.sync.dma_start_transpose(
                            vT[:, 0, :],
                            vt_tmp[0:64, j * 128:(j + 1) * 128])
                        nc.sync.dma_start_transpose(
                            vT[:, 1, :],
                            vt_tmp[64:128, j * 128:(j + 1) * 128])
                        nc.vector.tensor_copy(
                            v_s[:, tb, :].rearrange("p (h c) -> p h c",
                                                    h=2)[:, :, 0:64], vT)
                    chunk_state.pop(t)

            def proj_unit(ccs, tgs):
                """Transposed projection: stationary wp column chunk cc is
                reused across all moving token groups tgs."""
                for cc in ccs:
                    ccsl = slice(cc * 128, (cc + 1) * 128)
                    for tg in tgs:
                        tgsl = slice(tg * 512, (tg + 1) * 512)
                        py = psum.tile([128, 512], f32, tag="pework",
                                       bufs=2, name="proj")
                        nc.tensor.matmul(py, wp_sb[:, ccsl],
                                         gatedT_s[:, tgsl],
                                         start=True, stop=True)
                        yt = y_pool.tile([128, 512], bf16, tag="yt")
                        nc.vector.tensor_copy(yt, py)
                        nc.sync.dma_start(out=y[ccsl, tgsl], in_=yt)

            def proj_blocks(tbs):
                """Baseline-orientation projection: stationary gatedT token
                block, moving wp halves; y stays [T, C]."""
                for tb in tbs:
                    yt = y_pool.tile([128, 1024], bf16, tag="ytb")
                    for j in range(2):
                        py = psum.tile([128, 512], f32, tag="pework",
                                       bufs=2, name="proj")
                        nc.tensor.matmul(py,
                                         gatedT_s[:, tb * 128:(tb + 1) * 128],
                                         wp_sb[:, j * 512:(j + 1) * 512],
                                         start=True, stop=True)
                        nc.vector.tensor_copy(yt[:, j * 512:(j + 1) * 512],
                                              py)
                    nc.sync.dma_start(out=y[tb * 128:(tb + 1) * 128, :],
                                      in_=yt)

            def section(b, nh, jj, slot_work):
                """Attention for queries (b, nh*1024 + jj*512 .. +512), both
                heads, all 2048 keys.  Scores h0/h1 go out as tile_position
                (0,0)/(64,0) row-tile pairs (concurrent on the PE array);
                attn@v runs as two bank-contiguous 8-matmul DR chains after
                all 16 exps.  slot_work: callables interleaved into the
                section."""
                nsl = slice(b * N + nh * 1024 + jj * 512,
                            b * N + nh * 1024 + (jj + 1) * 512)
                psv = [psum.tile([65, 512], f32, tag=f"psv{h}",
                                 name=f"psv{h}") for h in range(2)] \
                    if stage >= 3 else None
                ptts = []
                for mbp in range(8):
                    for mi in range(2):
                        mb = mbp * 2 + mi
                        msl = slice(b * N + mb * 128, b * N + mb * 128 + 128)
                        ps = psum.tile([128, 2, 512], f32, tag="scores",
                                       bufs=2, name="scores")
                        nc.tensor.matmul(ps[:, 0, :], kpsT_s[0:64, msl],
                                         qT_s[0:64, nsl],
                                         start=True, stop=True)
                        nc.tensor.matmul(ps[:, 1, :], kpsT_s[64:128, msl],
                                         qT_s[64:128, nsl],
                                         start=True, stop=True)
                        if mi == 0:
                            ptts.append(pt_pool.tile([128, 2, 2, 512], fp8,
                                                     tag="pt", name="ptt",
                                                     bufs=3))
                        nc.scalar.activation(ptts[mbp][:, mi, :, :], ps, Exp,
                                             scale=SCALE)
                    if stage < 3:
                        sk = small.tile([1, 2, 2, 512], fp8, tag="sink")
                        nc.gpsimd.tensor_copy(sk, ptts[mbp][0:1])
                    else:
                        pr = b * NB + mbp * 2
                        for h in range(2):
                            nc.tensor.matmul(
                                psv[h],
                                v_s[:, pr:pr + 2, h * 80:h * 80 + 65],
                                ptts[mbp][:, :, h, :],
                                start=(mbp == 0), stop=(mbp == 7),
                                perf_mode=DR)
                    if mbp in (1, 3, 5) and slot_work:
                        slot_work.pop(0)()
                if stage < 3:
                    while slot_work:
                        slot_work.pop(0)()
                    return
                # normalize + gate both heads
                for h in range(2):
                    hsl = slice(h * 64, h * 64 + 64)
                    sacc = small.tile([65, 512], f32, tag="sacc")
                    nc.vector.tensor_copy(sacc, psv[h])
                    if norm == "off":
                        # perf probe only: skip the division (wrong math)
                        if h == 0:
                            nc.vector.tensor_mul(gatedT_s[hsl, nsl],
                                                 sacc[0:64, :],
                                                 gT_s[hsl, nsl])
                        else:
                            tmp2 = small.tile([128, 512], bf16, tag="tmp2")
                            nc.sync.dma_start(out=tmp2[64:128, :],
                                              in_=sacc[0:64, :])
                            nc.vector.tensor_mul(gatedT_s[hsl, nsl],
                                                 tmp2[64:128, :],
                                                 gT_s[hsl, nsl])
                        continue
                    # partition_broadcast only works from partition 0 on HW,
                    # so DMA the denominator row down to partition 0 first.
                    d0 = small.tile([1, 512], f32, tag="d0")
                    nc.sync.dma_start(out=d0, in_=sacc[64:65, :])
                    rs = small.tile([1, 512], f32, tag="rs")
                    nc.vector.reciprocal(rs, d0)
                    rb = small.tile([64, 512], f32, tag="rb")
                    nc.gpsimd.partition_broadcast(rb, rs)
                    tmp = small.tile([128, 512], bf16, tag="tmp")
                    eng = nc.vector if norm == "dve" else nc.gpsimd
                    eng.tensor_mul(tmp[0:64, :], sacc[0:64, :], rb)
                    if h == 0:
                        eng.tensor_mul(gatedT_s[hsl, nsl], tmp[0:64, :],
                                       gT_s[hsl, nsl])
                    else:
                        tmp2 = small.tile([128, 512], bf16, tag="tmp2")
                        nc.sync.dma_start(out=tmp2[64:128, :],
                                          in_=tmp[0:64, :])
                        eng.tensor_mul(gatedT_s[hsl, nsl],
                                       tmp2[64:128, :], gT_s[hsl, nsl])
                if stage < 4:
                    sk = small.tile([1, 512], bf16, tag="sink2")
                    nc.gpsimd.tensor_copy(sk, gatedT_s[0:1, nsl])
                while slot_work:
                    slot_work.pop(0)()

            # Steady-state schedule per rep (sections b0 S0-S3, b1 S4-S7):
            #   S0-S3 slack: phase A chunks 4-7 for THIS rep's b1
            #                + proj of PREV rep's b1 (tg 4-7)
            #   S4-S7 slack: phase A chunks 0-3 for NEXT rep's b0
            #                + proj of THIS rep's b0 (tg 0-3)
            def phase_a_units(chunks):
                """DMA prefetch runs two chunks ahead of the PE chains."""
                units = [lambda t=chunks[0]: phase_a_dma(t),
                         lambda t=chunks[1]: phase_a_dma(t)]
                for i, t in enumerate(chunks):
                    if i + 2 < len(chunks):
                        units.append(
                            lambda t2=chunks[i + 2]: phase_a_dma(t2))
                    units += [lambda t=t, p=p: phase_a_part(t, p)
                              for p in range(4)]
                return units

            for _rep in range(reps):
                first = _rep == 0
                last = _rep == reps - 1
                if first:
                    for u in phase_a_units((0, 1, 2, 3)):
                        u()
                if proj_t:
                    proj_b0 = [lambda c=c: proj_unit((2 * c, 2 * c + 1),
                                                     range(0, 4))
                               for c in range(4)]
                    proj_b1 = [lambda c=c: proj_unit((2 * c, 2 * c + 1),
                                                     range(4, 8))
                               for c in range(4)]
                else:
                    proj_b0 = [lambda r=r: proj_blocks(r)
                               for r in (range(0, 4), range(4, 8),
                                         range(8, 12), range(12, 16))]
                    proj_b1 = [lambda r=r: proj_blocks(r)
                               for r in (range(16, 20), range(20, 24),
                                         range(24, 28), range(28, 32))]
                b0_work = phase_a_units((4, 5, 6, 7))
                if not first and stage >= 4:
                    b0_work += proj_b1
                b1_work = []
                if not last:
                    b1_work += phase_a_units((0, 1, 2, 3))
                if stage >= 4:
                    b1_work += proj_b0
                secs = [(0, nh, jj) for nh in range(2) for jj in range(2)] + \
                       [(1, nh, jj) for nh in range(2) for jj in range(2)]
                for i, (b, nh, jj) in enumerate(secs):
                    work = b0_work if b == 0 else b1_work
                    k = i % 4
                    n_slots = -(-len(work) // (4 - k))
                    section(b, nh, jj, [work.pop(0)
                                        for _ in range(min(n_slots,
                                                           len(work)))])
            if stage >= 4:
                if proj_t:
                    proj_unit(range(8), range(4, 8))
                else:
                    proj_blocks(range(16, 32))

    nc.compile()
    return nc


def _get_nc():
    if "nc" not in _cache:
        _cache["nc"] = _build_program()
    return _cache["nc"]


def _get_exec():
    """Compile once; cache a persistent sharded executable."""
    if "exec" in _cache:
        return _cache["exec"]
    import jax
    from jax.experimental.shard_map import shard_map
    from jax.sharding import Mesh, PartitionSpec
    from concourse import mybir
    from concourse.bass2jax import (_bass_exec_p, install_neuronx_cc_hook,
                                    partition_id_tensor)

    nc = _get_nc()
    install_neuronx_cc_hook()
    partition_name = (nc.partition_id_tensor.name
                      if nc.partition_id_tensor else None)
    in_names, out_names, out_avals = [], [], []
    for alloc in nc.m.functions[0].allocations:
        if not isinstance(alloc, mybir.MemoryLocationSet):
            continue
        name = alloc.memorylocations[0].name
        if alloc.kind == "ExternalInput":
            if name != partition_name:
                in_names.append(name)
        elif alloc.kind == "ExternalOutput":
            out_names.append(name)
            out_avals.append(jax.core.ShapedArray(
                tuple(alloc.tensor_shape), mybir.dt.np(alloc.dtype)))
    n_params, n_outs = len(in_names), len(out_names)
    bind_in_names = tuple(in_names + out_names +
                          ([partition_name] if partition_name else []))

    def _body(*args):
        operands = list(args)
        if partition_name is not None:
            operands.append(partition_id_tensor())
        outs = _bass_exec_p.bind(
            *operands,
            out_avals=tuple(out_avals),
            in_names=bind_in_names,
            out_names=tuple(out_names),
            lowering_input_output_aliases=(),
            sim_require_finite=True,
            sim_require_nnan=True,
            nc=nc,
        )
        return tuple(outs)

    devices = jax.devices()[:NCORES]
    mesh = Mesh(np.asarray(devices), ("core",))
    in_specs = (PartitionSpec("core"),) * (n_params + n_outs)
    out_specs = (PartitionSpec("core"),) * n_outs
    sharded = jax.jit(shard_map(_body, mesh=mesh, in_specs=in_specs,
                                out_specs=out_specs, check_rep=False),
                      keep_unused=True)
    zeros_dev = [
        jax.device_put(
            np.zeros((NCORES * a.shape[0], *a.shape[1:]), a.dtype),
            jax.sharding.NamedSharding(mesh, PartitionSpec("core")))
        for a in out_avals]
    y_shape = out_avals[out_names.index("y")].shape
    if y_shape[0] * NCORES == NCORES * C and y_shape == (C, T):
        reduce_fn = jax.jit(
            lambda a: a.reshape(NCORES, C, T).astype(jax.numpy.float32)
            .sum(axis=0).T)
    else:
        reduce_fn = jax.jit(
            lambda a: a.reshape(NCORES, T, C).astype(jax.numpy.float32)
            .sum(axis=0))
    ex = {"fn": sharded, "in_names": in_names, "out_names": out_names,
          "out_avals": out_avals, "mesh": mesh, "zeros_dev": zeros_dev,
          "spec": PartitionSpec("core"), "reduce": reduce_fn}
    _cache["exec"] = ex
    return ex


def _make_in_maps(x, e, W_qkv, W_s, W_gate, W_proj):
    import ml_dtypes
    bf = ml_dtypes.bfloat16
    xT = np.ascontiguousarray(
        np.asarray(x, np.float32).reshape(T, C).T).astype(bf)
    eT = np.ascontiguousarray(
        np.asarray(e, np.float32).reshape(T, C).T).astype(bf)
    in_maps = []
    for c in range(NCORES):
        fs = slice(F * c, F * (c + 1))
        in_maps.append({
            "xT": xT,
            "eT": eT,
            "wq": np.ascontiguousarray(W_qkv[:, fs]).astype(bf),
            "wk": np.ascontiguousarray(W_qkv[:, C:][:, fs]).astype(bf),
            "wv": np.ascontiguousarray(W_qkv[:, 2 * C:][:, fs]).astype(bf),
            "ws": np.ascontiguousarray(W_s[:, fs]).astype(bf),
            "wg": np.ascontiguousarray(W_gate[:, fs]).astype(bf),
            "wp": np.ascontiguousarray(W_proj[fs, :]).astype(bf),
        })
    return in_maps


def kernel(x, e, W_qkv, W_s, W_gate, W_proj, b_proj):
    ex = _get_exec()
    in_maps = _make_in_maps(np.asarray(x), np.asarray(e), np.asarray(W_qkv),
                            np.asarray(W_s), np.asarray(W_gate),
                            np.asarray(W_proj))
    concat_in = [
        np.concatenate([np.asarray(in_maps[c][name])
                        for c in range(NCORES)], axis=0)
        for name in ex["in_names"]]
    out = ex["fn"](*concat_in, *ex["zeros_dev"])
    iy = ex["out_names"].index("y")
    y_sum = np.asarray(ex["reduce"](out[iy]))   # cross-core partial sum, [T,C]
    y_sum = y_sum + np.asarray(b_proj, dtype=np.float32)
    return y_sum.reshape(B, N, C).astype(np.float32)


# revision 3
# speedup vs baseline: 1.3137x; 1.1095x over previous
"""Trainium2 Bass kernel v3 for nn_Attention_48610439856262.

Gated attention block:
    qkv = x @ W_qkv ; gate = x @ W_gate ; s = e @ W_s (added to k)
    attn = softmax(q @ (k+s).T * D**-0.5) ; out = (attn @ v) * gate
    y = out @ W_proj + b_proj

Sharding (8 cores, tensor-parallel over heads): core c owns heads
{2c, 2c+1}.  Each core computes its 128 feature columns for q/k+s/v/gate,
runs attention for 2 heads, and writes the partial projection
y_c = gated_c @ W_proj[128c:128c+128, :]; the host sums partials + b_proj.

v3 changes vs v2 (427us HW steady-state):
  * scores via 2x PE row tiling: h0 (SBUF partitions 0-63) and h1 (64-127)
    issue back-to-back as tile_position (0,0)/(64,0) matmuls into separate
    PSUM banks of one [128, 2, 512] tile -> concurrent on the array.
  * one exp covers both heads' scores tile ([128, 1024] ACT instr).
  * attnv accumulates the full 2048-key range into psv (per section
    (b, nh, jj)) -> no spill/merge round-trip.
  * phase A runs chunk PAIRS sharing each stationary weight load (halves
    LDWEIGHTS traffic, which HW pays serially).
  * proj transposed: stationary = W_proj column chunk (loaded once per 4-8
    moving token groups), output yT [C, T]; host transposes.
"""

import numpy as np

B, N, C, H, D = 2, 2048, 1024, 16, 64
T = B * N              # 4096 tokens
NCORES = 8
F = 128                # feature columns per core (2 heads x 64)
KC = C // 128          # 8 contraction chunks
TB = T // 128          # 32 token blocks
NB = N // 128          # 16 key blocks per sequence
SCALE = D ** -0.5

_cache: dict = {}


def _build_program(reps=1, stage=4, norm="dve", drain_act=False,
                   sp_lite=False):
    import concourse.bacc as bacc
    import concourse.tile as tile
    from concourse import mybir

    f32 = mybir.dt.float32
    bf16 = mybir.dt.bfloat16
    fp8 = mybir.dt.float8e4
    DR = mybir.MatmulPerfMode.DoubleRow
    Exp = mybir.ActivationFunctionType.Exp

    nc = bacc.Bacc("TRN2", target_bir_lowering=False, debug=False,
                   num_devices=NCORES)

    xT = nc.dram_tensor("xT", [C, T], bf16, kind="ExternalInput").ap()
    eT = nc.dram_tensor("eT", [C, T], bf16, kind="ExternalInput").ap()
    wq = nc.dram_tensor("wq", [C, F], bf16, kind="ExternalInput").ap()
    wk = nc.dram_tensor("wk", [C, F], bf16, kind="ExternalInput").ap()
    wv = nc.dram_tensor("wv", [C, F], bf16, kind="ExternalInput").ap()
    ws = nc.dram_tensor("ws", [C, F], bf16, kind="ExternalInput").ap()
    wg = nc.dram_tensor("wg", [C, F], bf16, kind="ExternalInput").ap()
    wp = nc.dram_tensor("wp", [F, C], bf16, kind="ExternalInput").ap()
    y = nc.dram_tensor("y", [T, C], bf16, kind="ExternalOutput").ap()

    with tile.TileContext(nc) as tc:
        with tc.tile_pool(name="persist", bufs=1) as persist, \
             tc.tile_pool(name="psum", bufs=1, space="PSUM") as psum, \
             tc.tile_pool(name="xa", bufs=4) as xa_pool, \
             tc.tile_pool(name="ea", bufs=4) as ea_pool, \
             tc.tile_pool(name="vt", bufs=3) as vt_pool, \
             tc.tile_pool(name="pt", bufs=3) as pt_pool, \
             tc.tile_pool(name="small", bufs=4) as small, \
             tc.tile_pool(name="yout", bufs=4) as y_pool:
            # Weights, contraction-chunked: [128 k-part, KC, 128 cols]
            w_sb = {}
            for name, src in (("wq", wq), ("wk", wk), ("wv", wv),
                              ("ws", ws), ("wg", wg)):
                t_ = persist.tile([128, KC, F], bf16, tag=name, name=f"w_{name}")
                nc.sync.dma_start(out=t_,
                                  in_=src.rearrange("(k p) f -> p k f", p=128))
                w_sb[name] = t_
            wp_sb = persist.tile([F, C], bf16, tag="wp")
            nc.sync.dma_start(out=wp_sb, in_=wp)

            qT_s = persist.tile([128, T], bf16, tag="qT")
            kpsT_s = persist.tile([128, T], bf16, tag="kpsT")
            gT_s = persist.tile([128, T], bf16, tag="gT")
            gatedT_s = persist.tile([128, T], bf16, tag="gatedT")
            # v_aug per head: [v(64) | ones | pad(15)] x 2 heads -> 160/blk.
            # attn@v out partitions 0-63 = head dims, partition 64 = softmax
            # denominator.  160B block stride satisfies DoubleRow step%16.
            v_s = persist.tile([128, TB, 160], fp8, tag="v")
            if sp_lite:
                nc.vector.memset(v_s, 0.02)
            nc.vector.memset(v_s[:, :, 64], 1.0)
            nc.vector.memset(v_s[:, :, 144], 1.0)

            # PSUM (8 banks): scores [128,2,512] x 2 bufs = 4 | psv0,psv1 = 2
            # | pwork [128,2,512] = 2 (phase-A acc + proj; one generation,
            # halves alternate via subtile dep tracking so proj pairs can
            # drain both banks in a single DVE copy)

            chunk_state = {}
            pwork = psum.tile([128, 2, 512], f32, tag="pework", name="pwork")
            pw_state = {"i": 0}

            def next_half():
                i = pw_state["i"]
                pw_state["i"] = i ^ 1
                return pwork[:, i, :]

            def phase_a_dma(t):
                """Prefetch chunk t's x/e slabs (issued ahead of the PE
                chains so the first matmul never waits on HBM)."""
                sl = slice(t * 512, (t + 1) * 512)
                xt = xa_pool.tile([128, KC, 512], bf16, tag="xt",
                                  name=f"xt{t}")
                nc.sync.dma_start(
                    out=xt,
                    in_=xT[:, sl].rearrange("(k p) t -> p k t", p=128))
                et = ea_pool.tile([128, KC, 512], bf16, tag="et",
                                  name=f"et{t}")
                nc.sync.dma_start(
                    out=et,
                    in_=eT[:, sl].rearrange("(k p) t -> p k t", p=128))
                chunk_state[t] = (xt, et)

            def phase_a_part(t, part):
                """One quarter of chunk t's projections.  Chains stay
                bank-contiguous (HW pays heavily for interleaved PSUM
                accumulation groups).  part 0: q | 1: k+s | 2: gate
                | 3: v + transposes."""
                sl = slice(t * 512, (t + 1) * 512)
                xt, et = chunk_state[t]
                if part == 3:
                    # v computed TRANSPOSED on the PE: stationary = x token
                    # block (lhsT), moving = W_v -> out [tokens, vcols] lands
                    # token-major, exactly v_s's layout.  Kills the 8 DMA
                    # transposes + 4 small copies per chunk.
                    acc4 = next_half()
                    for j in range(4):
                        jsl = slice(j * 128, (j + 1) * 128)
                        for k in range(KC):
                            nc.tensor.matmul(acc4[:, jsl],
                                             xt[:, k, jsl],
                                             w_sb["wv"][:, k, :],
                                             start=(k == 0),
                                             stop=(k == KC - 1))
                    nc.vector.tensor_copy(
                        v_s[:, t * 4:(t + 1) * 4, :].rearrange(
                            "p b (h c) -> p b h c", h=2)[:, :, :, 0:64],
                        acc4.rearrange("p (b h c) -> p b h c", b=4, h=2))
                    chunk_state.pop(t)
                    return
                wname, dst = (("wq", qT_s), ("wk", kpsT_s),
                              ("wg", gT_s))[part]
                acc = next_half()
                for k in range(KC):
                    nc.tensor.matmul(acc, w_sb[wname][:, k, :], xt[:, k, :],
                                     start=(k == 0),
                                     stop=(part != 1 and k == KC - 1))
                if part == 1:
                    for k in range(KC):
                        nc.tensor.matmul(acc, w_sb["ws"][:, k, :],
                                         et[:, k, :],
                                         start=False, stop=(k == KC - 1))
                if drain_act and part == 1:
                    nc.scalar.copy(dst[:, sl], acc)
                else:
                    nc.vector.tensor_copy(dst[:, sl], acc)

            def proj_pair(tb2):
                """Projection of token blocks (2*tb2, 2*tb2+1); one merged
                y DMA per pair (halves the SP issue count)."""
                yt = y_pool.tile([128, 2, 1024], bf16, tag="ytb")
                for a in range(2):
                    tb = tb2 * 2 + a
                    pw_state["i"] = 0    # align so j maps to half j
                    for j in range(2):
                        py = next_half()
                        nc.tensor.matmul(py,
                                         gatedT_s[:, tb * 128:(tb + 1) * 128],
                                         wp_sb[:, j * 512:(j + 1) * 512],
                                         start=True, stop=True)
                    # both halves drained in ONE DVE copy
                    nc.vector.tensor_copy(
                        yt[:, a, :].rearrange("p (j q) -> p j q", j=2),
                        pwork)
                nc.sync.dma_start(
                    out=y[tb2 * 256:(tb2 + 1) * 256, :].rearrange(
                        "(a p) c -> p a c", p=128),
                    in_=yt)

            def section(b, nh, jj, slot_work):
                """Attention for queries (b, nh*1024 + jj*512 .. +512), both
                heads, all 2048 keys.  Scores h0/h1 go out as tile_position
                (0,0)/(64,0) row-tile pairs (concurrent on the PE array);
                attn@v runs as two bank-contiguous 8-matmul DR chains after
                all 16 exps.  slot_work: callables interleaved into the
                section."""
                nsl = slice(b * N + nh * 1024 + jj * 512,
                            b * N + nh * 1024 + (jj + 1) * 512)
                psv = [psum.tile([65, 512], f32, tag=f"psv{h}",
                                 name=f"psv{h}") for h in range(2)] \
                    if stage >= 3 else None
                ptts = []
                for mbp in range(8):
                    for mi in range(2):
                        mb = mbp * 2 + mi
                        msl = slice(b * N + mb * 128, b * N + mb * 128 + 128)
                        ps = psum.tile([128, 2, 512], f32, tag="scores",
                                       bufs=2, name="scores")
                        nc.tensor.matmul(ps[:, 0, :], kpsT_s[0:64, msl],
                                         qT_s[0:64, nsl],
                                         start=True, stop=True)
                        nc.tensor.matmul(ps[:, 1, :], kpsT_s[64:128, msl],
                                         qT_s[64:128, nsl],
                                         start=True, stop=True)
                        if mi == 0:
                            ptts.append(pt_pool.tile([128, 2, 2, 512], fp8,
                                                     tag="pt", name="ptt",
                                                     bufs=3))
                        nc.scalar.activation(ptts[mbp][:, mi, :, :], ps, Exp,
                                             scale=SCALE)
                    if stage < 3:
                        sk = small.tile([1, 2, 2, 512], fp8, tag="sink")
                        nc.gpsimd.tensor_copy(sk, ptts[mbp][0:1])
                    else:
                        pr = b * NB + mbp * 2
                        for h in range(2):
                            nc.tensor.matmul(
                                psv[h],
                                v_s[:, pr:pr + 2, h * 80:h * 80 + 65],
                                ptts[mbp][:, :, h, :],
                                start=(mbp == 0), stop=(mbp == 7),
                                perf_mode=DR)
                    if mbp >= 1 and slot_work:
                        slot_work.pop(0)()
                if stage < 3:
                    while slot_work:
                        slot_work.pop(0)()
                    return
                # normalize + gate both heads
                for h in range(2):
                    hsl = slice(h * 64, h * 64 + 64)
                    sacc = small.tile([65, 512], f32, tag="sacc")
                    nc.vector.tensor_copy(sacc, psv[h])
                    if norm == "off":
                        # perf probe only: skip the division (wrong math)
                        if h == 0:
                            nc.vector.tensor_mul(gatedT_s[hsl, nsl],
                                                 sacc[0:64, :],
                                                 gT_s[hsl, nsl])
                        else:
                            tmpb = small.tile([128, 512], bf16, tag="tmp")
                            nc.vector.tensor_copy(tmpb[0:64, :],
                                                  sacc[0:64, :])
                            tmp2 = small.tile([128, 512], bf16, tag="tmp2")
                            nc.sync.dma_start(out=tmp2[64:128, :],
                                              in_=tmpb[0:64, :])
                            nc.vector.tensor_mul(gatedT_s[hsl, nsl],
                                                 tmp2[64:128, :],
                                                 gT_s[hsl, nsl])
                        continue
                    # partition_broadcast only works from partition 0 on HW,
                    # so DMA the denominator row down to partition 0 first.
                    d0 = small.tile([1, 512], f32, tag="d0")
                    nc.sync.dma_start(out=d0, in_=sacc[64:65, :])
                    rs = small.tile([1, 512], f32, tag="rs")
                    nc.vector.reciprocal(rs, d0)
                    rb = small.tile([64, 512], f32, tag="rb")
                    nc.gpsimd.partition_broadcast(rb, rs)
                    tmp = small.tile([128, 512], bf16, tag="tmp")
                    eng = nc.vector if norm == "dve" else nc.gpsimd
                    eng.tensor_mul(tmp[0:64, :], sacc[0:64, :], rb)
                    if h == 0:
                        eng.tensor_mul(gatedT_s[hsl, nsl], tmp[0:64, :],
                                       gT_s[hsl, nsl])
                    else:
                        tmp2 = small.tile([128, 512], bf16, tag="tmp2")
                        nc.sync.dma_start(out=tmp2[64:128, :],
                                          in_=tmp[0:64, :])
                        eng.tensor_mul(gatedT_s[hsl, nsl],
                                       tmp2[64:128, :], gT_s[hsl, nsl])
                if stage < 4:
                    sk = small.tile([1, 512], bf16, tag="sink2")
                    nc.vector.tensor_copy(sk, gatedT_s[0:1, nsl])
                while slot_work:
                    slot_work.pop(0)()

            # Steady-state schedule per rep (sections b0 S0-S3, b1 S4-S7):
            #   S0-S3 slack: phase A chunks 4-7 for THIS rep's b1
            #                + proj of PREV rep's b1 (tg 4-7)
            #   S4-S7 slack: phase A chunks 0-3 for NEXT rep's b0
            #                + proj of THIS rep's b0 (tg 0-3)
            def phase_a_units(chunks, proj_tb2s=()):
                """DMA prefetch runs two chunks ahead of the PE chains.
                Every other PE chain part is paired with one projection
                pair: the proj matmuls' PSUM drains hide under the
                adjacent chain's compute (and vice versa)."""
                units = [lambda t=chunks[0]: phase_a_dma(t),
                         lambda t=chunks[1]: phase_a_dma(t)]
                proj_tb2s = list(proj_tb2s)
                for i, t in enumerate(chunks):
                    if i + 2 < len(chunks):
                        units.append(
                            lambda t2=chunks[i + 2]: phase_a_dma(t2))
                    for p in range(4):
                        units.append(lambda t=t, p=p: phase_a_part(t, p))
                        if p % 2 == 1 and proj_tb2s:
                            units.append(lambda tb2=proj_tb2s.pop(0):
                                         proj_pair(tb2))
                return units

            for _rep in range(reps):
                first = _rep == 0
                last = _rep == reps - 1
                if first:
                    for u in phase_a_units((0, 1, 2, 3)):
                        u()
                b0_work = phase_a_units(
                    (4, 5, 6, 7),
                    range(8, 16) if (not first and stage >= 4) else ())
                b1_work = []
                if not last:
                    b1_work += phase_a_units(
                        (0, 1, 2, 3),
                        range(0, 8) if stage >= 4 else ())
                elif stage >= 4:
                    b1_work += [lambda tb2=tb2: proj_pair(tb2)
                                for tb2 in range(0, 8)]
                secs = [(0, nh, jj) for nh in range(2) for jj in range(2)] + \
                       [(1, nh, jj) for nh in range(2) for jj in range(2)]
                for i, (b, nh, jj) in enumerate(secs):
                    work = b0_work if b == 0 else b1_work
                    k = i % 4
                    n_slots = -(-len(work) // (4 - k))
                    section(b, nh, jj, [work.pop(0)
                                        for _ in range(min(n_slots,
                                                           len(work)))])
            if stage >= 4:
                for tb2 in range(8, 16):
                    proj_pair(tb2)

    nc.compile()
    return nc


def _get_nc():
    if "nc" not in _cache:
        _cache["nc"] = _build_program()
    return _cache["nc"]


def _get_exec():
    """Compile once; cache a persistent sharded executable."""
    if "exec" in _cache:
        return _cache["exec"]
    import jax
    from jax.experimental.shard_map import shard_map
    from jax.sharding import Mesh, PartitionSpec
    from concourse import mybir
    from concourse.bass2jax import (_bass_exec_p, install_neuronx_cc_hook,
                                    partition_id_tensor)

    nc = _get_nc()
    install_neuronx_cc_hook()
    partition_name = (nc.partition_id_tensor.name
                      if nc.partition_id_tensor else None)
    in_names, out_names, out_avals = [], [], []
    for alloc in nc.m.functions[0].allocations:
        if not isinstance(alloc, mybir.MemoryLocationSet):
            continue
        name = alloc.memorylocations[0].name
        if alloc.kind == "ExternalInput":
            if name != partition_name:
                in_names.append(name)
        elif alloc.kind == "ExternalOutput":
            out_names.append(name)
            out_avals.append(jax.core.ShapedArray(
                tuple(alloc.tensor_shape), mybir.dt.np(alloc.dtype)))
    n_params, n_outs = len(in_names), len(out_names)
    bind_in_names = tuple(in_names + out_names +
                          ([partition_name] if partition_name else []))

    def _body(*args):
        operands = list(args)
        if partition_name is not None:
            operands.append(partition_id_tensor())
        outs = _bass_exec_p.bind(
            *operands,
            out_avals=tuple(out_avals),
            in_names=bind_in_names,
            out_names=tuple(out_names),
            lowering_input_output_aliases=(),
            sim_require_finite=True,
            sim_require_nnan=True,
            nc=nc,
        )
        return tuple(outs)

    devices = jax.devices()[:NCORES]
    mesh = Mesh(np.asarray(devices), ("core",))
    in_specs = (PartitionSpec("core"),) * (n_params + n_outs)
    out_specs = (PartitionSpec("core"),) * n_outs
    sharded = jax.jit(shard_map(_body, mesh=mesh, in_specs=in_specs,
                                out_specs=out_specs, check_rep=False),
                      keep_unused=True)
    zeros_dev = [
        jax.device_put(
            np.zeros((NCORES * a.shape[0], *a.shape[1:]), a.dtype),
            jax.sharding.NamedSharding(mesh, PartitionSpec("core")))
        for a in out_avals]
    y_shape = out_avals[out_names.index("y")].shape
    if y_shape[0] * NCORES == NCORES * C and y_shape == (C, T):
        reduce_fn = jax.jit(
            lambda a: a.reshape(NCORES, C, T).astype(jax.numpy.float32)
            .sum(axis=0).T)
    else:
        reduce_fn = jax.jit(
            lambda a: a.reshape(NCORES, T, C).astype(jax.numpy.float32)
            .sum(axis=0))
    ex = {"fn": sharded, "in_names": in_names, "out_names": out_names,
          "out_avals": out_avals, "mesh": mesh, "zeros_dev": zeros_dev,
          "spec": PartitionSpec("core"), "reduce": reduce_fn}
    _cache["exec"] = ex
    return ex


def _make_in_maps(x, e, W_qkv, W_s, W_gate, W_proj):
    import ml_dtypes
    bf = ml_dtypes.bfloat16
    xT = np.ascontiguousarray(
        np.asarray(x, np.float32).reshape(T, C).T).astype(bf)
    eT = np.ascontiguousarray(
        np.asarray(e, np.float32).reshape(T, C).T).astype(bf)
    in_maps = []
    for c in range(NCORES):
        fs = slice(F * c, F * (c + 1))
        in_maps.append({
            "xT": xT,
            "eT": eT,
            "wq": np.ascontiguousarray(W_qkv[:, fs]).astype(bf),
            "wk": np.ascontiguousarray(W_qkv[:, C:][:, fs]).astype(bf),
            "wv": np.ascontiguousarray(W_qkv[:, 2 * C:][:, fs]).astype(bf),
            "ws": np.ascontiguousarray(W_s[:, fs]).astype(bf),
            "wg": np.ascontiguousarray(W_gate[:, fs]).astype(bf),
            "wp": np.ascontiguousarray(W_proj[fs, :]).astype(bf),
        })
    return in_maps


def kernel(x, e, W_qkv, W_s, W_gate, W_proj, b_proj):
    ex = _get_exec()
    in_maps = _make_in_maps(np.asarray(x), np.asarray(e), np.asarray(W_qkv),
                            np.asarray(W_s), np.asarray(W_gate),
                            np.asarray(W_proj))
    concat_in = [
        np.concatenate([np.asarray(in_maps[c][name])
                        for c in range(NCORES)], axis=0)
        for name in ex["in_names"]]
    out = ex["fn"](*concat_in, *ex["zeros_dev"])
    iy = ex["out_names"].index("y")
    y_sum = np.asarray(ex["reduce"](out[iy]))   # cross-core partial sum, [T,C]
    y_sum = y_sum + np.asarray(b_proj, dtype=np.float32)
    return y_sum.reshape(B, N, C).astype(np.float32)
